# revision 1
# baseline (speedup 1.0000x reference)
"""ConfidenceBiasedCrossAttention Trainium2 kernel (8 NeuronCores).

Sharding (Megatron-style, per spec hint): data-parallel over B (2) x
head-parallel over 4 head-groups of 4 heads (256 channels) -> 8 cores.
Each core computes q/k/v projections for its 256 channels, biased
softmax attention for its 4 heads, and a partial output projection
(rows of Wo). Host sums the 4 partials per batch and adds the bias.

The V-projection bias is folded into the host-side bias: softmax
weights sum to 1, so attn @ (v + bv) = attn @ v + bv, and
sum_g bv_g @ Wo_g.T = Wv_b @ Wo_w.T is added on the host (exact).

Device pipeline per core (all matmuls in fp32r = fast fp32):
  - PE-transpose Q/K/V blocks and weight slices (f32r transpose mode)
  - qT/kT [256ch, L] and v [Lk, 256ch(+ones col)] projections
  - logitsT [Lk-chunk, Lq] = k_h qT_h; exp(0.125*x + V_bias[k]) on ACT
  - [attn_out.T | denom] accumulated over Lk chunks in PSUM via the
    ones column; normalize with reciprocal + K=1 broadcast matmul
  - partial out = attnT.T @ WoT accumulated over the 256 channels

PSUM budget (8 banks): pmm [128,256]x2 (transposes + proj matmuls,
1 bank each) + lg [128,1024]x2 (logits/bcast/Wo, 2 banks each) +
acc [128,1024]x1 (attn accumulator, 2 banks) = 8. Separate tags keep
the attention stream decoupled from the projection pipeline so the
scheduler can overlap them.
"""

import numpy as np

import concourse.bacc as bacc
import concourse.mybir as mybir
import concourse.tile as tile
from concourse import bass_utils
from concourse.masks import make_identity

F32 = mybir.dt.float32
F32R = mybir.dt.float32r
AF = mybir.ActivationFunctionType
MUL = mybir.AluOpType.mult

P = 128
C = 1024
D = 64
LQ = 1024
LK = 4096
CS = 256          # channels per core (4 heads)
NH = 4            # heads per core
SCALE = 1.0 / 8.0
BLK = 256         # Lk/Lq rows per processing block
NKB = LK // BLK   # 16
NQB = LQ // BLK   # 4
NCH = LK // P     # 32 Lk chunks of 128

TRANSPOSE_F32R = True  # f32r transpose mode: 1.5 cyc/row vs 2.0 for fp32


def _transpose_block(nc, ps, ident, dst, srcs):
    """Transpose [128,128] blocks of src (natural [row, ch]) into dst
    [128(ch-chunk), len*128(rows)] via PSUM."""
    pt = ps.tile([P, len(srcs) * P], F32R if TRANSPOSE_F32R else F32,
                 tag="pmm", bufs=2)
    for a, s in enumerate(srcs):
        nc.tensor.transpose(pt[:, a * P : (a + 1) * P], s, ident)
    nc.vector.tensor_copy(dst, pt if TRANSPOSE_F32R else pt)


def build_nc():
    nc = bacc.Bacc("TRN2", target_bir_lowering=False, debug=False, num_devices=8)
    XDT = F32R if TRANSPOSE_F32R else F32
    qb_d = nc.dram_tensor("Qb", [LQ, C], F32, kind="ExternalInput").ap()
    kb_d = nc.dram_tensor("Kb", [LK, C], F32, kind="ExternalInput").ap()
    vb_d = nc.dram_tensor("Vb", [LK, C], F32, kind="ExternalInput").ap()
    vbias_d = nc.dram_tensor("vbias", [P, NCH], F32, kind="ExternalInput").ap()
    wq_d = nc.dram_tensor("wq", [CS, C], F32, kind="ExternalInput").ap()
    wk_d = nc.dram_tensor("wk", [CS, C], F32, kind="ExternalInput").ap()
    wv_d = nc.dram_tensor("wv", [CS, C], F32, kind="ExternalInput").ap()
    wo_d = nc.dram_tensor("wo", [C, CS], F32, kind="ExternalInput").ap()
    bq_d = nc.dram_tensor("bq", [P, 2], F32, kind="ExternalInput").ap()
    bk_d = nc.dram_tensor("bk", [P, 2], F32, kind="ExternalInput").ap()
    out_d = nc.dram_tensor("out", [LQ, C], F32, kind="ExternalOutput").ap()

    def dram_x(ap):
        return ap.bitcast(F32R) if TRANSPOSE_F32R else ap

    with tile.TileContext(nc) as tc:
        with (
            tc.tile_pool(name="pers", bufs=1) as pers,
            tc.tile_pool(name="sb", bufs=1) as sb,
            tc.tile_pool(name="ps", bufs=2, space="PSUM") as ps,
        ):
            # ---- constants ----
            ident_f = pers.tile([P, P], F32)
            make_identity(nc, ident_f)
            if TRANSPOSE_F32R:
                ident = pers.tile([P, P], F32R)
                nc.vector.tensor_copy(ident, ident_f)
            else:
                ident = ident_f
            ones_f32 = pers.tile([P, 1], F32)
            nc.gpsimd.memset(ones_f32, 1.0)
            ones_r = pers.tile([1, P], F32R)
            nc.vector.tensor_copy(ones_r, ones_f32[0:1, :].to_broadcast([1, P]))
            vbias_sb = pers.tile([P, NCH], F32)
            nc.sync.dma_start(vbias_sb, vbias_d)
            bq_sb = pers.tile([P, 2], F32)
            nc.sync.dma_start(bq_sb, bq_d)
            bk_sb = pers.tile([P, 2], F32)
            nc.sync.dma_start(bk_sb, bk_d)

            # ---- persistent activations ----
            qT = pers.tile([P, 2, LQ], F32R)       # [ch%128, ch//128, Lq]
            kT = pers.tile([P, 2, LK], F32R)
            v65 = pers.tile([P, NCH, NH, D + 1], F32R)  # [k%128, chunk, h, v|1]
            attnT = pers.tile([P, 2, LQ], F32R)
            wqT = pers.tile([P, 8, CS], F32R)      # [cin%128, cin//128, cout]
            wkT = pers.tile([P, 8, CS], F32R)
            wvT = pers.tile([P, 8, CS], F32R)
            woT = pers.tile([P, 2, C], F32R)       # [ch%128, ch//128, cout]

            # ones column of v65 (denominator trick)
            nc.vector.tensor_copy(
                v65[:, :, :, D].rearrange("p a b -> p (a b)"),
                ones_f32.to_broadcast([P, NCH * NH]),
            )

            # ---- weight transposes ----
            for w_d_, wT in ((wq_d, wqT), (wk_d, wkT), (wv_d, wvT)):
                w_nat = sb.tile([P, 2, C], XDT, tag="wnat", bufs=2)
                nc.sync.dma_start(w_nat, dram_x(w_d_).rearrange("(t p) c -> p t c", p=P))
                for i in range(8):
                    _transpose_block(
                        nc, ps, ident, wT[:, i, :],
                        [w_nat[:, mt, i * P : (i + 1) * P] for mt in range(2)],
                    )
            wo_nat = sb.tile([P, 8, CS], XDT, tag="wnat", bufs=2)
            nc.sync.dma_start(wo_nat, dram_x(wo_d).rearrange("(t p) c -> p t c", p=P))
            for kc in range(2):
                pw = ps.tile([P, C], F32R if TRANSPOSE_F32R else F32, tag="lg")
                for j in range(8):
                    nc.tensor.transpose(
                        pw[:, j * P : (j + 1) * P],
                        wo_nat[:, j, kc * P : (kc + 1) * P],
                        ident,
                    )
                nc.vector.tensor_copy(woT[:, kc, :], pw)

            # ---- Q projection (4 blocks of 256 rows) ----
            def proj_block(x_d, blk, outs):
                xin = sb.tile([P, 2, C], XDT, tag="xin", bufs=2)
                nc.sync.dma_start(
                    xin,
                    dram_x(x_d[blk * BLK : (blk + 1) * BLK, :]).rearrange(
                        "(t p) c -> p t c", p=P
                    ),
                )
                xt = sb.tile([P, 8, BLK], F32R, tag="xt", bufs=2)
                for i in range(8):
                    _transpose_block(
                        nc, ps, ident, xt[:, i, :],
                        [xin[:, a, i * P : (i + 1) * P] for a in range(2)],
                    )
                outs(xt)

            def qk_out(wT, bias_sb, dstT, blk):
                def _o(xt):
                    for mt in range(2):
                        pq = ps.tile([P, BLK], F32, tag="pmm", bufs=2)
                        for i in range(8):
                            nc.tensor.matmul(
                                pq, wT[:, i, mt * P : (mt + 1) * P], xt[:, i, :],
                                start=(i == 0), stop=(i == 7),
                            )
                        nc.vector.tensor_scalar_add(
                            dstT[:, mt, blk * BLK : (blk + 1) * BLK], pq,
                            bias_sb[:, mt : mt + 1],
                        )
                return _o

            for blk in range(NQB):
                proj_block(qb_d, blk, qk_out(wqT, bq_sb, qT, blk))

            # ---- K & V projections, interleaved blocks of 256 rows ----
            def v_out(blk):
                def _o(xt):
                    for a in range(2):
                        pv = ps.tile([P, CS], F32, tag="pmm", bufs=2)
                        for i in range(8):
                            nc.tensor.matmul(
                                pv, xt[:, i, a * P : (a + 1) * P], wvT[:, i, :],
                                start=(i == 0), stop=(i == 7),
                            )
                        nc.vector.tensor_copy(
                            v65[:, blk * 2 + a, :, 0:D],
                            pv.rearrange("p (h d) -> p h d", d=D),
                        )
                return _o

            # ---- attention helpers ----
            def attn_chunk(h, c, po):
                ht, hp = h // 2, (h % 2) * D
                pl = ps.tile([P, LQ], F32, tag="lg", bufs=2, name="pl")
                for n in range(2):
                    nc.tensor.matmul(
                        pl[:, n * 512 : (n + 1) * 512],
                        kT[hp : hp + D, ht, c * P : (c + 1) * P],
                        qT[hp : hp + D, ht, n * 512 : (n + 1) * 512],
                        start=True, stop=True, tile_position=(hp, 0),
                    )
                eT = sb.tile([P, LQ], F32R, tag="exp", bufs=3, name="eT")
                nc.scalar.activation(
                    eT, pl, AF.Exp, bias=vbias_sb[:, c : c + 1], scale=SCALE
                )
                for n in range(2):
                    nc.tensor.matmul(
                        po[0 : D + 1, n * 512 : (n + 1) * 512],
                        v65[:, c, h, :],
                        eT[:, n * 512 : (n + 1) * 512],
                        start=(c == 0), stop=(c == NCH - 1),
                    )

            def attn_finish(h, po):
                ht, hp = h // 2, (h % 2) * D
                rec = sb.tile([1, LQ], F32R, tag="rec", name="rec")
                with nc.allow_low_precision(reason="softmax denom reciprocal"):
                    nc.vector.reciprocal(rec, po[D : D + 1, :])
                pb = ps.tile([P, LQ], F32, tag="lg", bufs=2, name="pb")
                for n in range(2):
                    nc.tensor.matmul(
                        pb[0:D, n * 512 : (n + 1) * 512],
                        ones_r[:, 0:D],
                        rec[:, n * 512 : (n + 1) * 512],
                        start=True, stop=True,
                    )
                bc = sb.tile([D, LQ], F32, tag="bc", name="bc")
                nc.vector.tensor_copy(bc, pb[0:D, :])
                nc.vector.tensor_tensor(
                    attnT[hp : hp + D, ht, :], po[0:D, :], bc, MUL
                )

            # head 0 streams behind the K/V projection blocks (its chunk c
            # only needs block c//2); heads 1-3 run as the tail.
            po0 = ps.tile([P, LQ], F32, tag="acc", bufs=1, name="po0")
            for blk in range(NKB):
                proj_block(kb_d, blk, qk_out(wkT, bk_sb, kT, blk))
                proj_block(vb_d, blk, v_out(blk))
                attn_chunk(0, 2 * blk, po0)
                attn_chunk(0, 2 * blk + 1, po0)
            attn_finish(0, po0)
            for h in range(1, NH):
                po = ps.tile([P, LQ], F32, tag="acc", bufs=1, name="po")
                for c in range(NCH):
                    attn_chunk(h, c, po)
                attn_finish(h, po)

            # ---- output projection (partial; host adds bias + reduces) ----
            for m in range(8):
                pw = ps.tile([P, C], F32, tag="lg", bufs=2)
                for kc in range(2):
                    for n in range(2):
                        nc.tensor.matmul(
                            pw[:, n * 512 : (n + 1) * 512],
                            attnT[:, kc, m * P : (m + 1) * P],
                            woT[:, kc, n * 512 : (n + 1) * 512],
                            start=(kc == 0), stop=(kc == 1),
                        )
                ob = sb.tile([P, C], F32, tag="ob", bufs=2)
                nc.vector.tensor_copy(ob, pw)
                nc.sync.dma_start(out_d[m * P : (m + 1) * P, :], ob)

    nc.compile()
    return nc


_NC = None


def _get_nc():
    global _NC
    if _NC is None:
        _NC = build_nc()
    return _NC


def shard_inputs(Q, K_in, V_in, V_bias, Wq_w, Wq_b, Wk_w, Wk_b, Wv_w, Wv_b, Wo_w, Wo_b):
    """Build the 8 per-core input dicts."""
    in_maps = []
    for core in range(8):
        b, g = core // 4, core % 4
        gs, ge = g * CS, (g + 1) * CS
        in_maps.append({
            "Qb": np.ascontiguousarray(Q[b]),
            "Kb": np.ascontiguousarray(K_in[b]),
            "Vb": np.ascontiguousarray(V_in[b]),
            "vbias": np.ascontiguousarray(V_bias[b].reshape(NCH, P).T),
            "wq": np.ascontiguousarray(Wq_w[gs:ge]),
            "wk": np.ascontiguousarray(Wk_w[gs:ge]),
            "wv": np.ascontiguousarray(Wv_w[gs:ge]),
            "wo": np.ascontiguousarray(Wo_w[:, gs:ge]),
            "bq": np.ascontiguousarray(Wq_b[gs:ge].reshape(2, P).T),
            "bk": np.ascontiguousarray(Wk_b[gs:ge].reshape(2, P).T),
        })
    return in_maps


def combine_outputs(results, Wv_b, Wo_w, Wo_b):
    """Sum the 4 head-group partials per batch; add output bias and the
    folded V-projection bias (attention weights sum to 1)."""
    bias = Wo_b + Wv_b @ Wo_w.T
    outs = np.stack([r["out"] for r in results]).reshape(2, 4, LQ, C)
    return (outs.sum(axis=1) + bias[None, None, :]).astype(np.float32)


def kernel(**inputs):
    nc = _get_nc()
    in_maps = shard_inputs(**inputs)
    res = bass_utils.run_bass_kernel_spmd(nc, in_maps, core_ids=list(range(8)))
    return combine_outputs(
        res.results,
        np.asarray(inputs["Wv_b"]),
        np.asarray(inputs["Wo_w"]),
        np.asarray(inputs["Wo_b"]),
    )


if __name__ == "__main__":
    rng = np.random.default_rng(0)
    ins = {
        "Q": rng.standard_normal((2, LQ, C), dtype=np.float32),
        "K_in": rng.standard_normal((2, LK, C), dtype=np.float32),
        "V_in": rng.standard_normal((2, LK, C), dtype=np.float32),
        "V_bias": rng.standard_normal((2, LK)).astype(np.float32),
        **{
            f"W{x}_w": (rng.standard_normal((C, C)) * 0.03).astype(np.float32)
            for x in "qkvo"
        },
        **{
            f"W{x}_b": (rng.standard_normal(C) * 0.03).astype(np.float32)
            for x in "qkvo"
        },
    }
    out = kernel(**ins)
    print("ok", out.shape, out.dtype)

